# revision 3
# baseline (speedup 1.0000x reference)
"""GCN layer (2 edge types, mean aggregation + self-loop) on 8 Trainium2 cores.

Math (per reference):
    m_t = segment_mean(h[src_t] @ Wt.T, dst_t)   for t in {1,2}
    out = relu(h @ Wl.T + bl + 0.5*(m1 + m2))

Key identity: linear commutes with gather+mean, so we aggregate raw h rows
(segment-mean) first and apply the 128x128 weights afterwards.

v3 design (slot-major, windowed gather, transposed accumulation):
  - Destination nodes partitioned contiguously across 8 cores; each core's
    12500 dst rows form 98 blocks of 128 ("slots" after a per-core
    permutation sorted by edge count, shared-profile SPMD).
  - Per (type, slot): ALL of its edges are gathered by ONE dma_gather call
    (<=7 chunks of 128 indices) out of a per-(type,slot) window of a
    per-core repacked node table hpk_c (window = that slot's unique src
    rows, so indices are int16 regardless of N).  Gather calls rotate
    across 4 SWDGE queues so descriptor-generation and SDMA drain pipeline.
  - Segment-mean per chunk via indicator matmul with the GATHERED ROWS as
    the stationary operand (lhsT=g, rhs=indicator), producing m^T [feat,
    dst] directly in PSUM and accumulating across all the slot's chunks —
    no SBUF accumulator and no separate transpose step.  The indicator is
    built pre-scaled by the dst inverse degree (fused is_equal*inv
    tensor_scalar), so the PSUM result is already the mean; the Scalar
    engine copies it to SBUF (bf16).
  - h rows are packed bf16 hi/lo ([*, 256]: cols 0:128 = bf16(h), 128:256 =
    bf16(h - hi)) giving 512B gather rows (full DMA line rate) and ~f32
    precision via two accumulating matmuls per chunk.
  - Final weight matmuls run in bf16 on slot pairs (256-wide outputs).
"""

import numpy as np
import ml_dtypes

BF16 = np.dtype(ml_dtypes.bfloat16)

N_NODES = 100000
HIDDEN = 128
N_CORES = 8
ROWS_PER_CORE = N_NODES // N_CORES  # 12500
KG = 14           # max chunks per dma_gather call (needs 32KB desc scratch)
N_TYPES = 2
PAD_DREL = 255.0  # dst_rel sentinel for padded edge slots -> indicator 0
NQ = 4            # SWDGE queues
SCRATCH = 32768   # SWDGE descriptor carveout (bytes/partition)


def _cdiv(a, b):
    return -(-a // b)


# ------------------------------------------------------------ host routing ---

def _route(srcs, dsts, rows_per_core, n_cores, n_nodes):
    """Host-side routing: per-core window tables + shared chunk schedule."""
    n_types = len(srcs)
    S_real = _cdiv(rows_per_core, 128)
    S = S_real + (S_real % 2)  # pad to even for slot-pairing

    # per-(core, type, block) counts
    counts = np.zeros((n_cores, n_types, S), np.int64)
    core_of, block_of, drel_of = [], [], []
    for t in range(n_types):
        dst = dsts[t].astype(np.int64)
        c = dst // rows_per_core
        dl = dst - c * rows_per_core
        b = dl // 128
        core_of.append(c)
        block_of.append(b)
        drel_of.append((dl - b * 128).astype(np.float32))
        np.add.at(counts, (c, t, b), 1)

    # shared per-core block->slot permutation (sorted by total count desc)
    key = counts.sum(axis=1)
    perms = np.argsort(-key, axis=1, kind="stable")
    inv_perms = np.argsort(perms, axis=1)

    sorted_counts = np.take_along_axis(counts, perms[:, None, :], axis=2)
    caps = np.maximum(_cdiv(sorted_counts, 128).max(axis=0), 1)  # [T, S]

    # chunk layout: slot-major, type alternating
    chunk_base = np.zeros((n_types, S), np.int64)
    pos = 0
    for s in range(S):
        for t in range(n_types):
            chunk_base[t, s] = pos
            pos += int(caps[t, s])
    n_chunks = pos

    # window sizes: max over cores of unique-src count per (t, s)
    ucnts = np.zeros((n_cores, n_types, S), np.int64)
    per_core_groups = []  # [c][t] -> (slot_sorted edge arrays)
    for c in range(n_cores):
        by_type = []
        for t in range(n_types):
            mask = core_of[t] == c
            e_idx = np.nonzero(mask)[0]
            slots = inv_perms[c][block_of[t][e_idx]]
            order = np.lexsort((srcs[t][e_idx], slots))
            e_idx = e_idx[order]
            slots = slots[order]
            e_src = srcs[t][e_idx].astype(np.int64)
            e_drel = drel_of[t][e_idx]
            e_dst = dsts[t][e_idx].astype(np.int64)
            starts = np.searchsorted(slots, np.arange(S + 1))
            for s in range(S):
                lo, hi = starts[s], starts[s + 1]
                if hi > lo:
                    seg = e_src[lo:hi]
                    ucnts[c, t, s] = 1 + int(np.count_nonzero(np.diff(seg)))
            by_type.append((e_src, e_drel, starts, e_dst))
        per_core_groups.append(by_type)

    wlen = np.maximum(ucnts.max(axis=0), 1)  # [T, S]
    assert wlen.max() <= 32767
    wbase = np.zeros((n_types, S), np.int64)
    pos = 0
    for s in range(S):
        for t in range(n_types):
            wbase[t, s] = pos
            pos += int(wlen[t, s])
    n_rows = pos

    # gather calls: per slot, both types merged (windows are adjacent),
    # pieces of <= KG chunks.  q0 is relative to chunk_base[0, s].
    calls = []  # (s, q0, w)
    for s in range(S):
        captot = int(caps[0, s]) + int(caps[1, s])
        q0 = 0
        while q0 < captot:
            w = min(KG, captot - q0)
            calls.append((s, q0, w))
            q0 += w

    invdeg = []
    for t in range(n_types):
        deg = np.bincount(dsts[t].astype(np.int64),
                          minlength=rows_per_core * n_cores)
        invdeg.append((1.0 / np.maximum(deg, 1)).astype(np.float32))

    per_core = []
    for c in range(n_cores):
        flat_idx = np.zeros(n_chunks * 128, np.int16)
        drel = np.full((128, n_chunks), PAD_DREL, np.float32)
        inve = np.zeros((128, n_chunks), np.float32)
        rows = np.zeros(n_rows, np.int64)
        for t in range(n_types):
            e_src, e_drel, starts, e_dst = per_core_groups[c][t]
            for s in range(S):
                lo, hi = starts[s], starts[s + 1]
                base = int(chunk_base[t, s]) * 128
                nslot = int(caps[t, s]) * 128
                wb, wl = int(wbase[t, s]), int(wlen[t, s])
                # call windows merge both types: t=1 indices are offset by
                # type-0's window length
                off = int(wlen[0, s]) if t == 1 else 0
                if hi > lo:
                    seg = e_src[lo:hi]
                    uniq, inverse = np.unique(seg, return_inverse=True)
                    rows[wb:wb + len(uniq)] = uniq
                    ucnt = len(uniq)
                    posn = base + np.arange(hi - lo)
                    flat_idx[posn] = (inverse + off).astype(np.int16)
                    drel[posn % 128, posn // 128] = e_drel[lo:hi]
                    inve[posn % 128, posn // 128] = \
                        invdeg[t][e_dst[lo:hi]]
                else:
                    ucnt = 1
                npad = nslot - (hi - lo)
                if npad > 0:
                    ppos = base + (hi - lo) + np.arange(npad)
                    flat_idx[ppos] = (np.arange(npad) % ucnt
                                      + off).astype(np.int16)

        # wrapped int16 index table per call: flat i -> partition i%16
        # (replicated across the 8 groups of 16 partitions), column i//16
        gidx_cols = []
        for (s, q0, w) in calls:
            c0 = int(chunk_base[0, s]) + q0
            seg = flat_idx[c0 * 128:(c0 + w) * 128]
            wrapped = seg.reshape(-1, 16).T  # [16, w*8]
            gidx_cols.append(np.tile(wrapped, (8, 1)))
        gidx = np.ascontiguousarray(np.concatenate(gidx_cols, axis=1))
        per_core.append(dict(gidx=gidx, drel=drel, inve=inve, perm=perms[c],
                             rows=rows))

    return dict(caps=caps, n_chunks=n_chunks, S=S, S_real=S_real,
                calls=calls, chunk_base=chunk_base, wbase=wbase, wlen=wlen,
                n_rows=n_rows, per_core=per_core)


# ------------------------------------------------------------ bass program ---

def _build_program(rt, n_nodes, n_cores, reps=1):
    """Build the SPMD bass program (shared by all cores)."""
    import concourse.bacc as bacc
    from concourse import mybir, tile, library_config

    caps, S = rt["caps"], rt["S"]
    n_chunks, calls, chunk_base = rt["n_chunks"], rt["calls"], rt["chunk_base"]
    wbase, wlen, n_rows = rt["wbase"], rt["wlen"], rt["n_rows"]
    n_types = caps.shape[0]
    capmax = int(caps.sum(axis=0).max())  # merged-call chunk count per slot
    F = HIDDEN
    nc = bacc.Bacc("TRN2", target_bir_lowering=False, debug=False,
                   num_devices=n_cores, num_swdge_queues=NQ,
                   dynamic_dma_scratch_size=SCRATCH)
    dt = mybir.dt

    hpk = nc.dram_tensor("hpk", [n_rows, 2 * F], dt.bfloat16,
                         kind="ExternalInput").ap()
    gidx_d = nc.dram_tensor("gidx", [128, n_chunks * 8], dt.int16,
                            kind="ExternalInput").ap()
    drel_d = nc.dram_tensor("drel", [128, n_chunks], dt.bfloat16,
                            kind="ExternalInput").ap()
    inve_d = nc.dram_tensor("inve", [128, n_chunks], dt.bfloat16,
                            kind="ExternalInput").ap()
    hot_d = nc.dram_tensor("hot", [128, S * 128], dt.bfloat16,
                           kind="ExternalInput").ap()
    w_d = [nc.dram_tensor(w, [128, 128], dt.bfloat16,
                          kind="ExternalInput").ap()
           for w in ("w1t", "w2t", "wlt")]
    blc_d = nc.dram_tensor("blc", [128, 1], dt.float32,
                           kind="ExternalInput").ap()
    iota_d = nc.dram_tensor("iota", [128, 128], dt.bfloat16,
                            kind="ExternalInput").ap()
    outT_d = nc.dram_tensor("outT", [128, S * 128], dt.bfloat16,
                            kind="ExternalOutput").ap()

    # calls grouped per slot
    calls_of = {}
    for (s, q0, w) in calls:
        calls_of.setdefault(s, []).append((q0, w))

    with tile.TileContext(nc) as tc:
        with (
            tc.tile_pool(name="const", bufs=1) as const_p,
            tc.tile_pool(name="gpool", bufs=10) as gpool,
            tc.tile_pool(name="ind", bufs=3) as ind_p,
            tc.tile_pool(name="inde", bufs=2) as inde_p,
            tc.tile_pool(name="mpair", bufs=2) as mt_p,
            tc.tile_pool(name="hot", bufs=2) as hot_p,
            tc.tile_pool(name="ostage", bufs=2) as o_p,
            tc.tile_pool(name="ps0", bufs=3, space="PSUM") as ps0_p,
            tc.tile_pool(name="ps1", bufs=3, space="PSUM") as ps1_p,
            tc.tile_pool(name="pso", bufs=2, space="PSUM") as pso_p,
        ):
            nc.gpsimd.load_library(library_config.mlp)
            gidx_s = const_p.tile([128, n_chunks * 8], dt.int16, name="gidx_s")
            nc.sync.dma_start(out=gidx_s[:], in_=gidx_d[:, :])
            drel_s = const_p.tile([128, n_chunks], dt.bfloat16, name="drel_s")
            nc.sync.dma_start(out=drel_s[:], in_=drel_d[:, :])
            inve_s = const_p.tile([128, n_chunks], dt.bfloat16, name="inve_s")
            nc.sync.dma_start(out=inve_s[:], in_=inve_d[:, :])
            w_s = []
            for i, wd in enumerate(w_d):
                wt = const_p.tile([128, 128], dt.bfloat16, tag=f"w{i}",
                                  name=f"ws{i}")
                nc.sync.dma_start(out=wt[:], in_=wd[:, :])
                w_s.append(wt)
            blc_s = const_p.tile([128, 1], dt.float32, name="blc_s")
            nc.sync.dma_start(out=blc_s[:], in_=blc_d[:, :])
            iota_s = const_p.tile([128, 128], dt.bfloat16, name="iota_s")
            nc.sync.dma_start(out=iota_s[:], in_=iota_d[:, :])

            bf16 = dt.bfloat16
            relu = mybir.ActivationFunctionType.Relu
            copyf = mybir.ActivationFunctionType.Copy
            iseq = mybir.AluOpType.is_equal
            mult = mybir.AluOpType.mult

            rr = [0]  # gather queue rotation

            for rep in range(reps):
                cur_mT = [None]

                def do_slot(s):
                    # one merged gather call per slot (both types; adjacent
                    # windows in hpk)
                    cb0 = int(chunk_base[0, s])
                    wb = int(wbase[0, s])
                    wl = int(wlen[0, s]) + int(wlen[1, s])
                    g = gpool.tile([128, capmax, 2 * F], dt.bfloat16,
                                   tag="g", name="g")
                    for (q0, w) in calls_of[s]:
                        nc.gpsimd.dma_gather(
                            g[:, q0:q0 + w, :], hpk[wb:wb + wl, :],
                            gidx_s[:, (cb0 + q0) * 8:(cb0 + q0 + w) * 8],
                            128 * w, 128 * w, 2 * F,
                            single_packet=False, queue_num=rr[0] % NQ)
                        rr[0] += 1
                    half = (s % 2) * 128
                    captot = int(caps[0, s]) + int(caps[1, s])
                    # batched indicator for the whole slot (both types):
                    # ind[e, j, d] = (drel[e, cb0+j] == d) * invdeg[dst]
                    inde = inde_p.tile([128, capmax * 128], dt.bfloat16,
                                       tag="inde", name="inde")
                    ind = ind_p.tile([128, capmax * 128], dt.bfloat16,
                                     tag="ind", name="ind")
                    de3 = drel_s[:, cb0:cb0 + captot].unsqueeze(-1) \
                        .broadcast_to([128, captot, 128])
                    iv3 = inve_s[:, cb0:cb0 + captot].unsqueeze(-1) \
                        .broadcast_to([128, captot, 128])
                    io3 = iota_s[:].unsqueeze(1) \
                        .broadcast_to([128, captot, 128])
                    inde3 = inde[:, :captot * 128].rearrange(
                        "p (j d) -> p j d", d=128)
                    ind3 = ind[:, :captot * 128].rearrange(
                        "p (j d) -> p j d", d=128)
                    nc.vector.tensor_tensor(out=inde3, in0=de3, in1=io3,
                                            op=iseq)
                    nc.vector.tensor_tensor(out=ind3, in0=inde3, in1=iv3,
                                            op=mult)
                    for t in range(n_types):
                        cap = int(caps[t, s])
                        cb = int(chunk_base[t, s])
                        goff = cb - cb0  # chunk offset within merged g tile
                        ps = (ps0_p if t == 0 else ps1_p).tile(
                            [128, 128], dt.float32, tag=f"ps{t}",
                            name=f"ps{t}")
                        for q in range(cap):
                            j = goff + q
                            # m^T accum: out[f, d] += g[e, f] * ind[e, d]
                            nc.tensor.matmul(
                                out=ps[:], lhsT=g[:, j, 0:F],
                                rhs=ind[:, j * 128:(j + 1) * 128],
                                start=(q == 0), stop=False)
                            nc.tensor.matmul(
                                out=ps[:], lhsT=g[:, j, F:2 * F],
                                rhs=ind[:, j * 128:(j + 1) * 128],
                                start=False,
                                stop=(q == cap - 1))
                        nc.scalar.activation(
                            out=cur_mT[0][t][:, half:half + 128], in_=ps[:],
                            func=copyf)

                cur_hot = [None]
                for s in range(S):
                    if s % 2 == 0:
                        cur_mT[0] = [
                            mt_p.tile([128, 256], bf16, tag=f"mt{t}",
                                      name=f"mt{t}") for t in range(n_types)]
                        q2 = s // 2
                        cur_hot[0] = hot_p.tile([128, 256], bf16, tag="hot",
                                                name="hot_t")
                        nc.sync.dma_start(
                            out=cur_hot[0][:],
                            in_=hot_d[:, q2 * 256:(q2 + 1) * 256])
                    do_slot(s)
                    if s % 2 == 1:
                        q2 = s // 2
                        hot_t = cur_hot[0]
                        pso = pso_p.tile([128, 256], dt.float32, tag="pso",
                                         name="pso")
                        nc.tensor.matmul(out=pso[:], lhsT=w_s[0][:],
                                         rhs=cur_mT[0][0][:],
                                         start=True, stop=False)
                        nc.tensor.matmul(out=pso[:], lhsT=w_s[1][:],
                                         rhs=cur_mT[0][1][:],
                                         start=False, stop=False)
                        nc.tensor.matmul(out=pso[:], lhsT=w_s[2][:],
                                         rhs=hot_t[:],
                                         start=False, stop=True)
                        ot = o_p.tile([128, 256], dt.bfloat16, tag="ot",
                                      name="ot")
                        nc.scalar.activation(out=ot[:], in_=pso[:], func=relu,
                                             bias=blc_s[:, 0:1])
                        nc.scalar.dma_start(
                            out=outT_d[:, q2 * 256:(q2 + 1) * 256], in_=ot[:])

    nc.compile()
    return nc


# ------------------------------------------------------------------ driver ---

def _prepare(h, src1, dst1, src2, dst2, W1, W2, Wl, bl,
             rows_per_core, n_cores):
    """Host-side packing. Returns (route, in_maps)."""
    h = np.asarray(h, np.float32)
    bl = np.asarray(bl, np.float32)
    srcs = [np.asarray(src1), np.asarray(src2)]
    dsts = [np.asarray(dst1), np.asarray(dst2)]
    n_nodes = h.shape[0]
    rt = _route(srcs, dsts, rows_per_core, n_cores, n_nodes)
    S = rt["S"]

    hi = h.astype(BF16)
    lo = (h - hi.astype(np.float32)).astype(BF16)
    hpk = np.concatenate([hi, lo], axis=1)  # [N, 256] bf16

    w1t = (0.5 * np.asarray(W1, np.float32).T).astype(BF16)
    w2t = (0.5 * np.asarray(W2, np.float32).T).astype(BF16)
    wlt = np.asarray(Wl, np.float32).T.astype(BF16)
    blc = bl.reshape(128, 1).copy()
    iota = np.broadcast_to(np.arange(128, dtype=np.float32), (128, 128))
    iota = np.ascontiguousarray(iota.astype(BF16))

    in_maps = []
    for c in range(n_cores):
        pc = rt["per_core"][c]
        rows = h[c * rows_per_core:(c + 1) * rows_per_core]
        pad = S * 128 - rows.shape[0]
        rows = np.pad(rows, ((0, pad), (0, 0)))
        blocks = rows.reshape(S, 128, HIDDEN)[pc["perm"]]
        hot = np.ascontiguousarray(
            blocks.transpose(2, 0, 1).reshape(HIDDEN, S * 128).astype(BF16))
        in_maps.append(dict(
            hpk=np.ascontiguousarray(hpk[pc["rows"]]),
            gidx=pc["gidx"], drel=pc["drel"].astype(BF16),
            inve=np.ascontiguousarray(pc["inve"].astype(BF16)),
            hot=hot, w1t=w1t, w2t=w2t, wlt=wlt, blc=blc, iota=iota,
        ))
    return rt, in_maps


def _postprocess(results, rt, rows_per_core, n_cores):
    n_nodes = rows_per_core * n_cores
    out = np.empty((n_nodes, HIDDEN), np.float32)
    for c in range(n_cores):
        outT = results[c]["outT"].astype(np.float32)  # [128, S*128]
        perm = rt["per_core"][c]["perm"]
        for s, b in enumerate(perm):
            lo_r = b * 128
            if lo_r >= rows_per_core:
                continue
            width = min(128, rows_per_core - lo_r)
            out[c * rows_per_core + lo_r:
                c * rows_per_core + lo_r + width] = \
                outT[:, s * 128:s * 128 + width].T
    return out


def kernel(h, src1, dst1, src2, dst2, W1, W2, Wl, bl, **kw):
    from concourse import bass_utils
    rt, in_maps = _prepare(h, src1, dst1, src2, dst2, W1, W2, Wl, bl,
                           ROWS_PER_CORE, N_CORES)
    nc = _build_program(rt, N_NODES, N_CORES)
    res = bass_utils.run_bass_kernel_spmd(
        nc, in_maps, core_ids=list(range(N_CORES)))
    return _postprocess(res.results, rt, ROWS_PER_CORE, N_CORES)


# revision 4
# speedup vs baseline: 1.1942x; 1.1942x over previous
"""GCN layer (2 edge types, mean aggregation + self-loop) on 8 Trainium2 cores.

Math (per reference):
    m_t = segment_mean(h[src_t] @ Wt.T, dst_t)   for t in {1,2}
    out = relu(h @ Wl.T + bl + 0.5*(m1 + m2))

Key identity: linear commutes with gather+mean, so we aggregate raw h rows
(segment-mean) first and apply the 128x128 weights afterwards.

v3 design (slot-major, windowed gather, transposed accumulation):
  - Destination nodes partitioned contiguously across 8 cores; each core's
    12500 dst rows form 98 blocks of 128 ("slots" after a per-core
    permutation sorted by edge count, shared-profile SPMD).
  - Per (type, slot): ALL of its edges are gathered by ONE dma_gather call
    (<=7 chunks of 128 indices) out of a per-(type,slot) window of a
    per-core repacked node table hpk_c (window = that slot's unique src
    rows, so indices are int16 regardless of N).  Gather calls rotate
    across 4 SWDGE queues so descriptor-generation and SDMA drain pipeline.
  - Segment-mean per chunk via indicator matmul with the GATHERED ROWS as
    the stationary operand (lhsT=g, rhs=indicator), producing m^T [feat,
    dst] directly in PSUM and accumulating across all the slot's chunks —
    no SBUF accumulator and no separate transpose step.  The indicator is
    built pre-scaled by the dst inverse degree (fused is_equal*inv
    tensor_scalar), so the PSUM result is already the mean; the Scalar
    engine copies it to SBUF (bf16).
  - h rows are packed bf16 hi/lo ([*, 256]: cols 0:128 = bf16(h), 128:256 =
    bf16(h - hi)) giving 512B gather rows (full DMA line rate) and ~f32
    precision via two accumulating matmuls per chunk.
  - Final weight matmuls run in bf16 on slot pairs (256-wide outputs).
"""

import os
import sys

import numpy as np
import ml_dtypes

_TRN_REPO = "/opt/trn_rl_repo"
if os.path.isdir(_TRN_REPO) and _TRN_REPO not in sys.path:
    sys.path.insert(0, _TRN_REPO)

BF16 = np.dtype(ml_dtypes.bfloat16)

N_NODES = 100000
HIDDEN = 128
N_CORES = 8
ROWS_PER_CORE = N_NODES // N_CORES  # 12500
KG = 14           # max chunks per dma_gather call (needs 32KB desc scratch)
N_TYPES = 2
PAD_DREL = 255.0  # dst_rel sentinel for padded edge slots -> indicator 0
NQ = 4            # SWDGE queues
SCRATCH = 32768   # SWDGE descriptor carveout (bytes/partition)


def _cdiv(a, b):
    return -(-a // b)


# ------------------------------------------------------------ host routing ---

def _route(srcs, dsts, rows_per_core, n_cores, n_nodes):
    """Host-side routing: per-core window tables + shared chunk schedule."""
    n_types = len(srcs)
    S_real = _cdiv(rows_per_core, 128)
    S = S_real + (S_real % 2)  # pad to even for slot-pairing

    # per-(core, type, block) counts
    counts = np.zeros((n_cores, n_types, S), np.int64)
    core_of, block_of, drel_of = [], [], []
    for t in range(n_types):
        dst = dsts[t].astype(np.int64)
        c = dst // rows_per_core
        dl = dst - c * rows_per_core
        b = dl // 128
        core_of.append(c)
        block_of.append(b)
        drel_of.append((dl - b * 128).astype(np.float32))
        np.add.at(counts, (c, t, b), 1)

    # shared per-core block->slot permutation (sorted by total count desc)
    key = counts.sum(axis=1)
    perms = np.argsort(-key, axis=1, kind="stable")
    inv_perms = np.argsort(perms, axis=1)

    sorted_counts = np.take_along_axis(counts, perms[:, None, :], axis=2)
    caps = np.maximum(_cdiv(sorted_counts, 128).max(axis=0), 1)  # [T, S]

    # chunk layout: slot-major, type alternating
    chunk_base = np.zeros((n_types, S), np.int64)
    pos = 0
    for s in range(S):
        for t in range(n_types):
            chunk_base[t, s] = pos
            pos += int(caps[t, s])
    n_chunks = pos

    # window sizes: max over cores of unique-src count per (t, s)
    ucnts = np.zeros((n_cores, n_types, S), np.int64)
    per_core_groups = []  # [c][t] -> (slot_sorted edge arrays)
    for c in range(n_cores):
        by_type = []
        for t in range(n_types):
            mask = core_of[t] == c
            e_idx = np.nonzero(mask)[0]
            slots = inv_perms[c][block_of[t][e_idx]]
            order = np.lexsort((srcs[t][e_idx], slots))
            e_idx = e_idx[order]
            slots = slots[order]
            e_src = srcs[t][e_idx].astype(np.int64)
            e_drel = drel_of[t][e_idx]
            e_dst = dsts[t][e_idx].astype(np.int64)
            starts = np.searchsorted(slots, np.arange(S + 1))
            for s in range(S):
                lo, hi = starts[s], starts[s + 1]
                if hi > lo:
                    seg = e_src[lo:hi]
                    ucnts[c, t, s] = 1 + int(np.count_nonzero(np.diff(seg)))
            by_type.append((e_src, e_drel, starts, e_dst))
        per_core_groups.append(by_type)

    wlen = np.maximum(ucnts.max(axis=0), 1)  # [T, S]
    assert wlen.max() <= 32767
    wbase = np.zeros((n_types, S), np.int64)
    pos = 0
    for s in range(S):
        for t in range(n_types):
            wbase[t, s] = pos
            pos += int(wlen[t, s])
    n_rows = pos

    # gather calls: per slot, both types merged (windows are adjacent),
    # pieces of <= KG chunks.  q0 is relative to chunk_base[0, s].
    calls = []  # (s, q0, w)
    for s in range(S):
        captot = int(caps[0, s]) + int(caps[1, s])
        q0 = 0
        while q0 < captot:
            w = min(KG, captot - q0)
            calls.append((s, q0, w))
            q0 += w

    invdeg = []
    for t in range(n_types):
        deg = np.bincount(dsts[t].astype(np.int64),
                          minlength=rows_per_core * n_cores)
        invdeg.append((1.0 / np.maximum(deg, 1)).astype(np.float32))

    per_core = []
    for c in range(n_cores):
        flat_idx = np.zeros(n_chunks * 128, np.int16)
        drel = np.full((128, n_chunks), PAD_DREL, np.float32)
        inve = np.zeros((128, n_chunks), np.float32)
        rows = np.zeros(n_rows, np.int64)
        for t in range(n_types):
            e_src, e_drel, starts, e_dst = per_core_groups[c][t]
            for s in range(S):
                lo, hi = starts[s], starts[s + 1]
                base = int(chunk_base[t, s]) * 128
                nslot = int(caps[t, s]) * 128
                wb, wl = int(wbase[t, s]), int(wlen[t, s])
                # call windows merge both types: t=1 indices are offset by
                # type-0's window length
                off = int(wlen[0, s]) if t == 1 else 0
                if hi > lo:
                    seg = e_src[lo:hi]
                    uniq, inverse = np.unique(seg, return_inverse=True)
                    rows[wb:wb + len(uniq)] = uniq
                    ucnt = len(uniq)
                    posn = base + np.arange(hi - lo)
                    flat_idx[posn] = (inverse + off).astype(np.int16)
                    drel[posn % 128, posn // 128] = e_drel[lo:hi]
                    inve[posn % 128, posn // 128] = \
                        invdeg[t][e_dst[lo:hi]]
                else:
                    ucnt = 1
                npad = nslot - (hi - lo)
                if npad > 0:
                    ppos = base + (hi - lo) + np.arange(npad)
                    flat_idx[ppos] = (np.arange(npad) % ucnt
                                      + off).astype(np.int16)

        # wrapped int16 index table per call: flat i -> partition i%16
        # (replicated across the 8 groups of 16 partitions), column i//16
        gidx_cols = []
        for (s, q0, w) in calls:
            c0 = int(chunk_base[0, s]) + q0
            seg = flat_idx[c0 * 128:(c0 + w) * 128]
            wrapped = seg.reshape(-1, 16).T  # [16, w*8]
            gidx_cols.append(np.tile(wrapped, (8, 1)))
        gidx = np.ascontiguousarray(np.concatenate(gidx_cols, axis=1))
        per_core.append(dict(gidx=gidx, drel=drel, inve=inve, perm=perms[c],
                             rows=rows))

    return dict(caps=caps, n_chunks=n_chunks, S=S, S_real=S_real,
                calls=calls, chunk_base=chunk_base, wbase=wbase, wlen=wlen,
                n_rows=n_rows, per_core=per_core)


# ------------------------------------------------------------ bass program ---

def _build_program(rt, n_nodes, n_cores, reps=1):
    """Build the SPMD bass program (shared by all cores)."""
    import concourse.bacc as bacc
    from concourse import mybir, tile, library_config

    caps, S = rt["caps"], rt["S"]
    n_chunks, calls, chunk_base = rt["n_chunks"], rt["calls"], rt["chunk_base"]
    wbase, wlen, n_rows = rt["wbase"], rt["wlen"], rt["n_rows"]
    n_types = caps.shape[0]
    capmax = int(caps.sum(axis=0).max())  # merged-call chunk count per slot
    F = HIDDEN
    nc = bacc.Bacc("TRN2", target_bir_lowering=False, debug=False,
                   num_devices=n_cores, num_swdge_queues=NQ,
                   dynamic_dma_scratch_size=SCRATCH)
    dt = mybir.dt

    hpk = nc.dram_tensor("hpk", [n_rows, 2 * F], dt.bfloat16,
                         kind="ExternalInput").ap()
    gidx_d = nc.dram_tensor("gidx", [128, n_chunks * 8], dt.int16,
                            kind="ExternalInput").ap()
    drel_d = nc.dram_tensor("drel", [128, n_chunks], dt.bfloat16,
                            kind="ExternalInput").ap()
    inve_d = nc.dram_tensor("inve", [128, n_chunks], dt.bfloat16,
                            kind="ExternalInput").ap()
    hot_d = nc.dram_tensor("hot", [128, S * 128], dt.bfloat16,
                           kind="ExternalInput").ap()
    w_d = [nc.dram_tensor(w, [128, 128], dt.bfloat16,
                          kind="ExternalInput").ap()
           for w in ("w1t", "w2t", "wlt")]
    blc_d = nc.dram_tensor("blc", [128, 1], dt.float32,
                           kind="ExternalInput").ap()
    iota_d = nc.dram_tensor("iota", [128, 128], dt.bfloat16,
                            kind="ExternalInput").ap()
    outT_d = nc.dram_tensor("outT", [128, S * 128], dt.bfloat16,
                            kind="ExternalOutput").ap()

    # calls grouped per slot
    calls_of = {}
    for (s, q0, w) in calls:
        calls_of.setdefault(s, []).append((q0, w))

    with tile.TileContext(nc) as tc:
        with (
            tc.tile_pool(name="const", bufs=1) as const_p,
            tc.tile_pool(name="gpool", bufs=10) as gpool,
            tc.tile_pool(name="ind", bufs=3) as ind_p,
            tc.tile_pool(name="inde", bufs=2) as inde_p,
            tc.tile_pool(name="mpair", bufs=2) as mt_p,
            tc.tile_pool(name="hot", bufs=2) as hot_p,
            tc.tile_pool(name="ostage", bufs=2) as o_p,
            tc.tile_pool(name="ps0", bufs=3, space="PSUM") as ps0_p,
            tc.tile_pool(name="ps1", bufs=3, space="PSUM") as ps1_p,
            tc.tile_pool(name="pso", bufs=2, space="PSUM") as pso_p,
        ):
            nc.gpsimd.load_library(library_config.mlp)
            gidx_s = const_p.tile([128, n_chunks * 8], dt.int16, name="gidx_s")
            nc.sync.dma_start(out=gidx_s[:], in_=gidx_d[:, :])
            drel_s = const_p.tile([128, n_chunks], dt.bfloat16, name="drel_s")
            nc.sync.dma_start(out=drel_s[:], in_=drel_d[:, :])
            inve_s = const_p.tile([128, n_chunks], dt.bfloat16, name="inve_s")
            nc.sync.dma_start(out=inve_s[:], in_=inve_d[:, :])
            w_s = []
            for i, wd in enumerate(w_d):
                wt = const_p.tile([128, 128], dt.bfloat16, tag=f"w{i}",
                                  name=f"ws{i}")
                nc.sync.dma_start(out=wt[:], in_=wd[:, :])
                w_s.append(wt)
            blc_s = const_p.tile([128, 1], dt.float32, name="blc_s")
            nc.sync.dma_start(out=blc_s[:], in_=blc_d[:, :])
            iota_s = const_p.tile([128, 128], dt.bfloat16, name="iota_s")
            nc.sync.dma_start(out=iota_s[:], in_=iota_d[:, :])

            bf16 = dt.bfloat16
            relu = mybir.ActivationFunctionType.Relu
            copyf = mybir.ActivationFunctionType.Copy
            iseq = mybir.AluOpType.is_equal
            mult = mybir.AluOpType.mult

            rr = [0]  # gather queue rotation

            for rep in range(reps):
                cur_mT = [None]

                def do_slot(s):
                    # one merged gather call per slot (both types; adjacent
                    # windows in hpk)
                    cb0 = int(chunk_base[0, s])
                    wb = int(wbase[0, s])
                    wl = int(wlen[0, s]) + int(wlen[1, s])
                    g = gpool.tile([128, capmax, 2 * F], dt.bfloat16,
                                   tag="g", name="g")
                    for (q0, w) in calls_of[s]:
                        nc.gpsimd.dma_gather(
                            g[:, q0:q0 + w, :], hpk[wb:wb + wl, :],
                            gidx_s[:, (cb0 + q0) * 8:(cb0 + q0 + w) * 8],
                            128 * w, 128 * w, 2 * F,
                            single_packet=False, queue_num=rr[0] % NQ)
                        rr[0] += 1
                    half = (s % 2) * 128
                    captot = int(caps[0, s]) + int(caps[1, s])
                    # batched indicator for the whole slot (both types):
                    # ind[e, j, d] = (drel[e, cb0+j] == d) * invdeg[dst]
                    inde = inde_p.tile([128, capmax * 128], dt.bfloat16,
                                       tag="inde", name="inde")
                    ind = ind_p.tile([128, capmax * 128], dt.bfloat16,
                                     tag="ind", name="ind")
                    de3 = drel_s[:, cb0:cb0 + captot].unsqueeze(-1) \
                        .broadcast_to([128, captot, 128])
                    iv3 = inve_s[:, cb0:cb0 + captot].unsqueeze(-1) \
                        .broadcast_to([128, captot, 128])
                    io3 = iota_s[:].unsqueeze(1) \
                        .broadcast_to([128, captot, 128])
                    inde3 = inde[:, :captot * 128].rearrange(
                        "p (j d) -> p j d", d=128)
                    ind3 = ind[:, :captot * 128].rearrange(
                        "p (j d) -> p j d", d=128)
                    nc.vector.tensor_tensor(out=inde3, in0=de3, in1=io3,
                                            op=iseq)
                    nc.vector.tensor_tensor(out=ind3, in0=inde3, in1=iv3,
                                            op=mult)
                    for t in range(n_types):
                        cap = int(caps[t, s])
                        cb = int(chunk_base[t, s])
                        goff = cb - cb0  # chunk offset within merged g tile
                        ps = (ps0_p if t == 0 else ps1_p).tile(
                            [128, 128], dt.float32, tag=f"ps{t}",
                            name=f"ps{t}")
                        for q in range(cap):
                            j = goff + q
                            # m^T accum: out[f, d] += g[e, f] * ind[e, d]
                            nc.tensor.matmul(
                                out=ps[:], lhsT=g[:, j, 0:F],
                                rhs=ind[:, j * 128:(j + 1) * 128],
                                start=(q == 0), stop=False)
                            nc.tensor.matmul(
                                out=ps[:], lhsT=g[:, j, F:2 * F],
                                rhs=ind[:, j * 128:(j + 1) * 128],
                                start=False,
                                stop=(q == cap - 1))
                        nc.scalar.activation(
                            out=cur_mT[0][t][:, half:half + 128], in_=ps[:],
                            func=copyf)

                for s in range(S):
                    if s % 2 == 0:
                        cur_mT[0] = [
                            mt_p.tile([128, 256], bf16, tag=f"mt{t}",
                                      name=f"mt{t}") for t in range(n_types)]
                    do_slot(s)
                    if s % 2 == 1:
                        q2 = s // 2
                        hot_t = hot_p.tile([128, 256], bf16, tag="hot",
                                           name="hot_t")
                        nc.sync.dma_start(
                            out=hot_t[:],
                            in_=hot_d[:, q2 * 256:(q2 + 1) * 256])
                        pso = pso_p.tile([128, 256], dt.float32, tag="pso",
                                         name="pso")
                        nc.tensor.matmul(out=pso[:], lhsT=w_s[0][:],
                                         rhs=cur_mT[0][0][:],
                                         start=True, stop=False)
                        nc.tensor.matmul(out=pso[:], lhsT=w_s[1][:],
                                         rhs=cur_mT[0][1][:],
                                         start=False, stop=False)
                        nc.tensor.matmul(out=pso[:], lhsT=w_s[2][:],
                                         rhs=hot_t[:],
                                         start=False, stop=True)
                        ot = o_p.tile([128, 256], dt.bfloat16, tag="ot",
                                      name="ot")
                        nc.scalar.activation(out=ot[:], in_=pso[:], func=relu,
                                             bias=blc_s[:, 0:1])
                        nc.sync.dma_start(
                            out=outT_d[:, q2 * 256:(q2 + 1) * 256], in_=ot[:])

    nc.compile()
    return nc


# ------------------------------------------------------------------ driver ---

def _prepare(h, src1, dst1, src2, dst2, W1, W2, Wl, bl,
             rows_per_core, n_cores):
    """Host-side packing. Returns (route, in_maps)."""
    h = np.asarray(h, np.float32)
    bl = np.asarray(bl, np.float32)
    srcs = [np.asarray(src1), np.asarray(src2)]
    dsts = [np.asarray(dst1), np.asarray(dst2)]
    n_nodes = h.shape[0]
    rt = _route(srcs, dsts, rows_per_core, n_cores, n_nodes)
    S = rt["S"]

    hi = h.astype(BF16)
    lo = (h - hi.astype(np.float32)).astype(BF16)
    hpk = np.concatenate([hi, lo], axis=1)  # [N, 256] bf16

    w1t = (0.5 * np.asarray(W1, np.float32).T).astype(BF16)
    w2t = (0.5 * np.asarray(W2, np.float32).T).astype(BF16)
    wlt = np.asarray(Wl, np.float32).T.astype(BF16)
    blc = bl.reshape(128, 1).copy()
    iota = np.broadcast_to(np.arange(128, dtype=np.float32), (128, 128))
    iota = np.ascontiguousarray(iota.astype(BF16))

    in_maps = []
    for c in range(n_cores):
        pc = rt["per_core"][c]
        rows = h[c * rows_per_core:(c + 1) * rows_per_core]
        pad = S * 128 - rows.shape[0]
        rows = np.pad(rows, ((0, pad), (0, 0)))
        blocks = rows.reshape(S, 128, HIDDEN)[pc["perm"]]
        hot = np.ascontiguousarray(
            blocks.transpose(2, 0, 1).reshape(HIDDEN, S * 128).astype(BF16))
        in_maps.append(dict(
            hpk=np.ascontiguousarray(hpk[pc["rows"]]),
            gidx=pc["gidx"], drel=pc["drel"].astype(BF16),
            inve=np.ascontiguousarray(pc["inve"].astype(BF16)),
            hot=hot, w1t=w1t, w2t=w2t, wlt=wlt, blc=blc, iota=iota,
        ))
    return rt, in_maps


def _postprocess(results, rt, rows_per_core, n_cores):
    n_nodes = rows_per_core * n_cores
    out = np.empty((n_nodes, HIDDEN), np.float32)
    for c in range(n_cores):
        outT = results[c]["outT"].astype(np.float32)  # [128, S*128]
        perm = rt["per_core"][c]["perm"]
        for s, b in enumerate(perm):
            lo_r = b * 128
            if lo_r >= rows_per_core:
                continue
            width = min(128, rows_per_core - lo_r)
            out[c * rows_per_core + lo_r:
                c * rows_per_core + lo_r + width] = \
                outT[:, s * 128:s * 128 + width].T
    return out


def kernel(h, src1, dst1, src2, dst2, W1, W2, Wl, bl, **kw):
    from concourse import bass_utils
    rt, in_maps = _prepare(h, src1, dst1, src2, dst2, W1, W2, Wl, bl,
                           ROWS_PER_CORE, N_CORES)
    nc = _build_program(rt, N_NODES, N_CORES)
    res = bass_utils.run_bass_kernel_spmd(
        nc, in_maps, core_ids=list(range(N_CORES)))
    return _postprocess(res.results, rt, ROWS_PER_CORE, N_CORES)


# revision 5
# speedup vs baseline: 1.2911x; 1.0811x over previous
"""GCN layer (2 edge types, mean aggregation + self-loop) on 8 Trainium2 cores.

Math (per reference):
    m_t = segment_mean(h[src_t] @ Wt.T, dst_t)   for t in {1,2}
    out = relu(h @ Wl.T + bl + 0.5*(m1 + m2))

Key identity: linear commutes with gather+mean, so we aggregate raw h rows
(segment-mean) first and apply the 128x128 weights afterwards.

v3 design (slot-major, windowed gather, transposed accumulation):
  - Destination nodes partitioned contiguously across 8 cores; each core's
    12500 dst rows form 98 blocks of 128 ("slots" after a per-core
    permutation sorted by edge count, shared-profile SPMD).
  - Per (type, slot): ALL of its edges are gathered by ONE dma_gather call
    (<=7 chunks of 128 indices) out of a per-(type,slot) window of a
    per-core repacked node table hpk_c (window = that slot's unique src
    rows, so indices are int16 regardless of N).  Gather calls rotate
    across 4 SWDGE queues so descriptor-generation and SDMA drain pipeline.
  - Segment-mean per chunk via indicator matmul with the GATHERED ROWS as
    the stationary operand (lhsT=g, rhs=indicator), producing m^T [feat,
    dst] directly in PSUM and accumulating across all the slot's chunks —
    no SBUF accumulator and no separate transpose step.  The indicator is
    built pre-scaled by the dst inverse degree (fused is_equal*inv
    tensor_scalar), so the PSUM result is already the mean; the Scalar
    engine copies it to SBUF (bf16).
  - h rows are packed bf16 hi/lo ([*, 256]: cols 0:128 = bf16(h), 128:256 =
    bf16(h - hi)) giving 512B gather rows (full DMA line rate) and ~f32
    precision via two accumulating matmuls per chunk.
  - Final weight matmuls run in bf16 on slot pairs (256-wide outputs).
"""

import os
import sys

import numpy as np
import ml_dtypes

_TRN_REPO = "/opt/trn_rl_repo"
if os.path.isdir(_TRN_REPO) and _TRN_REPO not in sys.path:
    sys.path.insert(0, _TRN_REPO)

BF16 = np.dtype(ml_dtypes.bfloat16)

N_NODES = 100000
HIDDEN = 128
N_CORES = 8
ROWS_PER_CORE = N_NODES // N_CORES  # 12500
KG = 14           # max chunks per dma_gather call (needs 32KB desc scratch)
N_TYPES = 2
PAD_DREL = 255.0  # dst_rel sentinel for padded edge slots -> indicator 0
NQ = 4            # SWDGE queues
SCRATCH = 32768   # SWDGE descriptor carveout (bytes/partition)


def _cdiv(a, b):
    return -(-a // b)


# ------------------------------------------------------------ host routing ---

def _route(srcs, dsts, rows_per_core, n_cores, n_nodes):
    """Host-side routing: per-core window tables + shared chunk schedule."""
    n_types = len(srcs)
    S_real = _cdiv(rows_per_core, 128)
    S = S_real + (S_real % 2)  # pad to even for slot-pairing

    # per-(core, type, block) counts
    counts = np.zeros((n_cores, n_types, S), np.int64)
    core_of, block_of, drel_of = [], [], []
    for t in range(n_types):
        dst = dsts[t].astype(np.int64)
        c = dst // rows_per_core
        dl = dst - c * rows_per_core
        b = dl // 128
        core_of.append(c)
        block_of.append(b)
        drel_of.append((dl - b * 128).astype(np.float32))
        np.add.at(counts, (c, t, b), 1)

    # shared per-core block->slot permutation (sorted by total count desc)
    key = counts.sum(axis=1)
    perms = np.argsort(-key, axis=1, kind="stable")
    inv_perms = np.argsort(perms, axis=1)

    sorted_counts = np.take_along_axis(counts, perms[:, None, :], axis=2)
    caps = np.maximum(_cdiv(sorted_counts, 128).max(axis=0), 1)  # [T, S]

    # chunk layout: slot-major, type alternating
    chunk_base = np.zeros((n_types, S), np.int64)
    pos = 0
    for s in range(S):
        for t in range(n_types):
            chunk_base[t, s] = pos
            pos += int(caps[t, s])
    n_chunks = pos

    # window sizes: max over cores of unique-src count per (t, s)
    ucnts = np.zeros((n_cores, n_types, S), np.int64)
    per_core_groups = []  # [c][t] -> (slot_sorted edge arrays)
    for c in range(n_cores):
        by_type = []
        for t in range(n_types):
            mask = core_of[t] == c
            e_idx = np.nonzero(mask)[0]
            slots = inv_perms[c][block_of[t][e_idx]]
            order = np.lexsort((srcs[t][e_idx], slots))
            e_idx = e_idx[order]
            slots = slots[order]
            e_src = srcs[t][e_idx].astype(np.int64)
            e_drel = drel_of[t][e_idx]
            e_dst = dsts[t][e_idx].astype(np.int64)
            starts = np.searchsorted(slots, np.arange(S + 1))
            for s in range(S):
                lo, hi = starts[s], starts[s + 1]
                if hi > lo:
                    seg = e_src[lo:hi]
                    ucnts[c, t, s] = 1 + int(np.count_nonzero(np.diff(seg)))
            by_type.append((e_src, e_drel, starts, e_dst))
        per_core_groups.append(by_type)

    wlen = np.maximum(ucnts.max(axis=0), 1)  # [T, S]
    assert wlen.max() <= 32767
    wbase = np.zeros((n_types, S), np.int64)
    pos = 0
    for s in range(S):
        for t in range(n_types):
            wbase[t, s] = pos
            pos += int(wlen[t, s])
    n_rows = pos

    # gather calls: per slot, both types merged (windows are adjacent),
    # pieces of <= KG chunks.  q0 is relative to chunk_base[0, s].
    calls = []  # (s, q0, w)
    for s in range(S):
        captot = int(caps[0, s]) + int(caps[1, s])
        q0 = 0
        while q0 < captot:
            w = min(KG, captot - q0)
            calls.append((s, q0, w))
            q0 += w

    invdeg = []
    for t in range(n_types):
        deg = np.bincount(dsts[t].astype(np.int64),
                          minlength=rows_per_core * n_cores)
        invdeg.append((1.0 / np.maximum(deg, 1)).astype(np.float32))

    per_core = []
    for c in range(n_cores):
        flat_idx = np.zeros(n_chunks * 128, np.int16)
        drel = np.full((128, n_chunks), PAD_DREL, np.float32)
        inve = np.zeros((128, n_chunks), np.float32)
        rows = np.zeros(n_rows, np.int64)
        for t in range(n_types):
            e_src, e_drel, starts, e_dst = per_core_groups[c][t]
            for s in range(S):
                lo, hi = starts[s], starts[s + 1]
                base = int(chunk_base[t, s]) * 128
                nslot = int(caps[t, s]) * 128
                wb, wl = int(wbase[t, s]), int(wlen[t, s])
                # call windows merge both types: t=1 indices are offset by
                # type-0's window length
                off = int(wlen[0, s]) if t == 1 else 0
                if hi > lo:
                    seg = e_src[lo:hi]
                    uniq, inverse = np.unique(seg, return_inverse=True)
                    rows[wb:wb + len(uniq)] = uniq
                    ucnt = len(uniq)
                    posn = base + np.arange(hi - lo)
                    flat_idx[posn] = (inverse + off).astype(np.int16)
                    drel[posn % 128, posn // 128] = e_drel[lo:hi]
                    inve[posn % 128, posn // 128] = \
                        invdeg[t][e_dst[lo:hi]]
                else:
                    ucnt = 1
                npad = nslot - (hi - lo)
                if npad > 0:
                    ppos = base + (hi - lo) + np.arange(npad)
                    flat_idx[ppos] = (np.arange(npad) % ucnt
                                      + off).astype(np.int16)

        # wrapped int16 index table per call: flat i -> partition i%16
        # (replicated across the 8 groups of 16 partitions), column i//16
        gidx_cols = []
        for (s, q0, w) in calls:
            c0 = int(chunk_base[0, s]) + q0
            seg = flat_idx[c0 * 128:(c0 + w) * 128]
            wrapped = seg.reshape(-1, 16).T  # [16, w*8]
            gidx_cols.append(np.tile(wrapped, (8, 1)))
        gidx = np.ascontiguousarray(np.concatenate(gidx_cols, axis=1))
        per_core.append(dict(gidx=gidx, drel=drel, inve=inve, perm=perms[c],
                             rows=rows))

    return dict(caps=caps, n_chunks=n_chunks, S=S, S_real=S_real,
                calls=calls, chunk_base=chunk_base, wbase=wbase, wlen=wlen,
                n_rows=n_rows, per_core=per_core)


# ------------------------------------------------------------ bass program ---

def _build_program(rt, n_nodes, n_cores, reps=1):
    """Build the SPMD bass program (shared by all cores)."""
    import concourse.bacc as bacc
    from concourse import mybir, tile, library_config

    caps, S = rt["caps"], rt["S"]
    n_chunks, calls, chunk_base = rt["n_chunks"], rt["calls"], rt["chunk_base"]
    wbase, wlen, n_rows = rt["wbase"], rt["wlen"], rt["n_rows"]
    n_types = caps.shape[0]
    capmax = int(caps.sum(axis=0).max())  # merged-call chunk count per slot
    F = HIDDEN
    nc = bacc.Bacc("TRN2", target_bir_lowering=False, debug=False,
                   num_devices=n_cores, num_swdge_queues=NQ,
                   dynamic_dma_scratch_size=SCRATCH)
    dt = mybir.dt

    hpk = nc.dram_tensor("hpk", [n_rows, 2 * F], dt.bfloat16,
                         kind="ExternalInput").ap()
    gidx_d = nc.dram_tensor("gidx", [128, n_chunks * 8], dt.int16,
                            kind="ExternalInput").ap()
    drel_d = nc.dram_tensor("drel", [128, n_chunks], dt.bfloat16,
                            kind="ExternalInput").ap()
    inve_d = nc.dram_tensor("inve", [128, n_chunks], dt.bfloat16,
                            kind="ExternalInput").ap()
    hot_d = nc.dram_tensor("hot", [128, S * 128], dt.bfloat16,
                           kind="ExternalInput").ap()
    w_d = [nc.dram_tensor(w, [128, 128], dt.bfloat16,
                          kind="ExternalInput").ap()
           for w in ("w1t", "w2t", "wlt")]
    blc_d = nc.dram_tensor("blc", [128, 1], dt.float32,
                           kind="ExternalInput").ap()
    iota_d = nc.dram_tensor("iota", [128, 128], dt.bfloat16,
                            kind="ExternalInput").ap()
    outT_d = nc.dram_tensor("outT", [128, S * 128], dt.bfloat16,
                            kind="ExternalOutput").ap()

    # calls grouped per slot
    calls_of = {}
    for (s, q0, w) in calls:
        calls_of.setdefault(s, []).append((q0, w))

    with tile.TileContext(nc) as tc:
        with (
            tc.tile_pool(name="const", bufs=1) as const_p,
            tc.tile_pool(name="gpool", bufs=12) as gpool,
            tc.tile_pool(name="ind", bufs=5) as ind_p,
            tc.tile_pool(name="inde", bufs=2) as inde_p,
            tc.tile_pool(name="mpair", bufs=2) as mt_p,
            tc.tile_pool(name="hot", bufs=2) as hot_p,
            tc.tile_pool(name="ostage", bufs=2) as o_p,
            tc.tile_pool(name="ps0", bufs=3, space="PSUM") as ps0_p,
            tc.tile_pool(name="ps1", bufs=3, space="PSUM") as ps1_p,
            tc.tile_pool(name="pso", bufs=2, space="PSUM") as pso_p,
        ):
            nc.gpsimd.load_library(library_config.mlp)
            gidx_s = const_p.tile([128, n_chunks * 8], dt.int16, name="gidx_s")
            nc.sync.dma_start(out=gidx_s[:], in_=gidx_d[:, :])
            drel_s = const_p.tile([128, n_chunks], dt.bfloat16, name="drel_s")
            nc.sync.dma_start(out=drel_s[:], in_=drel_d[:, :])
            inve_s = const_p.tile([128, n_chunks], dt.bfloat16, name="inve_s")
            nc.sync.dma_start(out=inve_s[:], in_=inve_d[:, :])
            w_s = []
            for i, wd in enumerate(w_d):
                wt = const_p.tile([128, 128], dt.bfloat16, tag=f"w{i}",
                                  name=f"ws{i}")
                nc.sync.dma_start(out=wt[:], in_=wd[:, :])
                w_s.append(wt)
            blc_s = const_p.tile([128, 1], dt.float32, name="blc_s")
            nc.sync.dma_start(out=blc_s[:], in_=blc_d[:, :])
            iota_s = const_p.tile([128, 128], dt.bfloat16, name="iota_s")
            nc.sync.dma_start(out=iota_s[:], in_=iota_d[:, :])

            bf16 = dt.bfloat16
            relu = mybir.ActivationFunctionType.Relu
            copyf = mybir.ActivationFunctionType.Copy
            iseq = mybir.AluOpType.is_equal
            mult = mybir.AluOpType.mult

            rr = [0]  # gather queue rotation

            for rep in range(reps):
                cur_mT = [None]

                def do_slot(s):
                    # one merged gather call per slot (both types; adjacent
                    # windows in hpk)
                    cb0 = int(chunk_base[0, s])
                    wb = int(wbase[0, s])
                    wl = int(wlen[0, s]) + int(wlen[1, s])
                    g = gpool.tile([128, capmax, 2 * F], dt.bfloat16,
                                   tag="g", name="g")
                    for (q0, w) in calls_of[s]:
                        nc.gpsimd.dma_gather(
                            g[:, q0:q0 + w, :], hpk[wb:wb + wl, :],
                            gidx_s[:, (cb0 + q0) * 8:(cb0 + q0 + w) * 8],
                            128 * w, 128 * w, 2 * F,
                            single_packet=False, queue_num=rr[0] % NQ)
                        rr[0] += 1
                    half = (s % 2) * 128
                    captot = int(caps[0, s]) + int(caps[1, s])
                    # batched indicator for the whole slot (both types):
                    # ind[e, j, d] = (drel[e, cb0+j] == d) * invdeg[dst]
                    inde = inde_p.tile([128, capmax * 128], dt.bfloat16,
                                       tag="inde", name="inde")
                    ind = ind_p.tile([128, capmax * 128], dt.bfloat16,
                                     tag="ind", name="ind")
                    de3 = drel_s[:, cb0:cb0 + captot].unsqueeze(-1) \
                        .broadcast_to([128, captot, 128])
                    iv3 = inve_s[:, cb0:cb0 + captot].unsqueeze(-1) \
                        .broadcast_to([128, captot, 128])
                    io3 = iota_s[:].unsqueeze(1) \
                        .broadcast_to([128, captot, 128])
                    inde3 = inde[:, :captot * 128].rearrange(
                        "p (j d) -> p j d", d=128)
                    ind3 = ind[:, :captot * 128].rearrange(
                        "p (j d) -> p j d", d=128)
                    nc.vector.tensor_tensor(out=inde3, in0=de3, in1=io3,
                                            op=iseq)
                    nc.vector.tensor_tensor(out=ind3, in0=inde3, in1=iv3,
                                            op=mult)
                    for t in range(n_types):
                        cap = int(caps[t, s])
                        cb = int(chunk_base[t, s])
                        goff = cb - cb0  # chunk offset within merged g tile
                        ps = (ps0_p if t == 0 else ps1_p).tile(
                            [128, 128], dt.float32, tag=f"ps{t}",
                            name=f"ps{t}")
                        for q in range(cap):
                            j = goff + q
                            # m^T accum: out[f, d] += g[e, f] * ind[e, d]
                            nc.tensor.matmul(
                                out=ps[:], lhsT=g[:, j, 0:F],
                                rhs=ind[:, j * 128:(j + 1) * 128],
                                start=(q == 0), stop=False)
                            nc.tensor.matmul(
                                out=ps[:], lhsT=g[:, j, F:2 * F],
                                rhs=ind[:, j * 128:(j + 1) * 128],
                                start=False,
                                stop=(q == cap - 1))
                        nc.scalar.activation(
                            out=cur_mT[0][t][:, half:half + 128], in_=ps[:],
                            func=copyf)

                for s in range(S):
                    if s % 2 == 0:
                        cur_mT[0] = [
                            mt_p.tile([128, 256], bf16, tag=f"mt{t}",
                                      name=f"mt{t}") for t in range(n_types)]
                    do_slot(s)
                    if s % 2 == 1:
                        q2 = s // 2
                        hot_t = hot_p.tile([128, 256], bf16, tag="hot",
                                           name="hot_t")
                        nc.sync.dma_start(
                            out=hot_t[:],
                            in_=hot_d[:, q2 * 256:(q2 + 1) * 256])
                        pso = pso_p.tile([128, 256], dt.float32, tag="pso",
                                         name="pso")
                        nc.tensor.matmul(out=pso[:], lhsT=w_s[0][:],
                                         rhs=cur_mT[0][0][:],
                                         start=True, stop=False)
                        nc.tensor.matmul(out=pso[:], lhsT=w_s[1][:],
                                         rhs=cur_mT[0][1][:],
                                         start=False, stop=False)
                        nc.tensor.matmul(out=pso[:], lhsT=w_s[2][:],
                                         rhs=hot_t[:],
                                         start=False, stop=True)
                        ot = o_p.tile([128, 256], dt.bfloat16, tag="ot",
                                      name="ot")
                        nc.scalar.activation(out=ot[:], in_=pso[:], func=relu,
                                             bias=blc_s[:, 0:1])
                        nc.sync.dma_start(
                            out=outT_d[:, q2 * 256:(q2 + 1) * 256], in_=ot[:])

    nc.compile()
    return nc


# ------------------------------------------------------------------ driver ---

def _prepare(h, src1, dst1, src2, dst2, W1, W2, Wl, bl,
             rows_per_core, n_cores):
    """Host-side packing. Returns (route, in_maps)."""
    h = np.asarray(h, np.float32)
    bl = np.asarray(bl, np.float32)
    srcs = [np.asarray(src1), np.asarray(src2)]
    dsts = [np.asarray(dst1), np.asarray(dst2)]
    n_nodes = h.shape[0]
    rt = _route(srcs, dsts, rows_per_core, n_cores, n_nodes)
    S = rt["S"]

    hi = h.astype(BF16)
    lo = (h - hi.astype(np.float32)).astype(BF16)
    hpk = np.concatenate([hi, lo], axis=1)  # [N, 256] bf16

    w1t = (0.5 * np.asarray(W1, np.float32).T).astype(BF16)
    w2t = (0.5 * np.asarray(W2, np.float32).T).astype(BF16)
    wlt = np.asarray(Wl, np.float32).T.astype(BF16)
    blc = bl.reshape(128, 1).copy()
    iota = np.broadcast_to(np.arange(128, dtype=np.float32), (128, 128))
    iota = np.ascontiguousarray(iota.astype(BF16))

    in_maps = []
    for c in range(n_cores):
        pc = rt["per_core"][c]
        rows = h[c * rows_per_core:(c + 1) * rows_per_core]
        pad = S * 128 - rows.shape[0]
        rows = np.pad(rows, ((0, pad), (0, 0)))
        blocks = rows.reshape(S, 128, HIDDEN)[pc["perm"]]
        hot = np.ascontiguousarray(
            blocks.transpose(2, 0, 1).reshape(HIDDEN, S * 128).astype(BF16))
        in_maps.append(dict(
            hpk=np.ascontiguousarray(hpk[pc["rows"]]),
            gidx=pc["gidx"], drel=pc["drel"].astype(BF16),
            inve=np.ascontiguousarray(pc["inve"].astype(BF16)),
            hot=hot, w1t=w1t, w2t=w2t, wlt=wlt, blc=blc, iota=iota,
        ))
    return rt, in_maps


def _postprocess(results, rt, rows_per_core, n_cores):
    n_nodes = rows_per_core * n_cores
    out = np.empty((n_nodes, HIDDEN), np.float32)
    for c in range(n_cores):
        outT = results[c]["outT"].astype(np.float32)  # [128, S*128]
        perm = rt["per_core"][c]["perm"]
        for s, b in enumerate(perm):
            lo_r = b * 128
            if lo_r >= rows_per_core:
                continue
            width = min(128, rows_per_core - lo_r)
            out[c * rows_per_core + lo_r:
                c * rows_per_core + lo_r + width] = \
                outT[:, s * 128:s * 128 + width].T
    return out


def kernel(h, src1, dst1, src2, dst2, W1, W2, Wl, bl, **kw):
    from concourse import bass_utils
    rt, in_maps = _prepare(h, src1, dst1, src2, dst2, W1, W2, Wl, bl,
                           ROWS_PER_CORE, N_CORES)
    nc = _build_program(rt, N_NODES, N_CORES)
    res = bass_utils.run_bass_kernel_spmd(
        nc, in_maps, core_ids=list(range(N_CORES)))
    return _postprocess(res.results, rt, ROWS_PER_CORE, N_CORES)
